# revision 4
# baseline (speedup 1.0000x reference)
"""Rowwise cosine-similarity kernel for Trainium2 (8 NeuronCores, SPMD).

Computes out[b, n] = sum_d(an * bn) where an, bn are L2-normalized rows of
a, b [16, 4096, 256] -> out [16, 4096].

Sharding: 65536 rows split across 8 cores (8192 rows/core). Per core the
row slice is viewed as [128 partitions, 64 subtiles, 256], i.e. row
p*64 + t lives at partition p, subtile t. Everything is contiguous DMA.

Per 256-wide subtile:
  P  = sum(a*b)  via DVE scalar_tensor_tensor (fused mul + accum-reduce)
  Sa = sum(a*a)  via ACT Square+accum_out or DVE stt (load-balanced)
  Sb = sum(b*b)  likewise
Finalize (batched [128, 64]): out = P * sqrt(1/(Sa*Sb)).
"""

import sys

if "/opt/trn_rl_repo" not in sys.path:
    sys.path.insert(0, "/opt/trn_rl_repo")

import numpy as np
import orjson

import concourse.bass as bass
import concourse.mybir as mybir
import concourse.tile as tile
from concourse import bass2jax, bass_utils

# ---------------------------------------------------------------------------
# Environment patches.
#
# 1. No cloud share in this sandbox: upload_artifacts would fail.
# 2. The walrus build here accepts at most ONE semaphore wait per
#    instruction; the Tile scheduler freely attaches several.  Post-process
#    the BIR before compiling: move surplus waits onto single-wait Drain
#    carrier instructions inserted just before the original instruction on
#    the same engine queue.
# ---------------------------------------------------------------------------

bass_utils.upload_artifacts = lambda tmpdir: ""

_MAX_WAITS = 1


def _split_bir_waits(bir_json: bytes) -> bytes:
    d = orjson.loads(bir_json)
    ctr = 0
    for fn in d.get("functions", []):
        for blk in fn.get("blocks", []):
            insts = blk.get("instructions")
            if not insts:
                continue
            out = []
            for inst in insts:
                si = inst.get("sync_info")
                waits = (si or {}).get("on_wait") or []
                if len(waits) > _MAX_WAITS:
                    surplus = waits[:-_MAX_WAITS]
                    si["on_wait"] = waits[-_MAX_WAITS:]
                    for i in range(0, len(surplus), _MAX_WAITS):
                        out.append(
                            {
                                "name": f"WSPL-{ctr}",
                                "opcode": "Drain",
                                "engine": inst["engine"],
                                "ins": [],
                                "outs": [],
                                "is_reset_sema": False,
                                "debug": inst.get("debug", 0),
                                "sync_info": {
                                    "on_wait": surplus[i : i + _MAX_WAITS],
                                    "on_update": [],
                                },
                            }
                        )
                        ctr += 1
                out.append(inst)
            blk["instructions"] = out
    return orjson.dumps(d)


_orig_compile_bir_kernel = bass_utils.compile_bir_kernel


def _patched_compile_bir_kernel(bir_json, tmpdir, neff_name="file.neff"):
    if isinstance(bir_json, str):
        bir_json = bir_json.encode()
    bir_json = _split_bir_waits(bir_json)
    return _orig_compile_bir_kernel(bir_json, tmpdir, neff_name=neff_name)


bass_utils.compile_bir_kernel = _patched_compile_bir_kernel
bass2jax.compile_bir_kernel = _patched_compile_bir_kernel

# ---------------------------------------------------------------------------
# Problem constants (hardcoded; kernel.py must be self-contained).
# ---------------------------------------------------------------------------

N_CORES = 8
B, N, D = 16, 4096, 256
ROWS = B * N                     # 65536
ROWS_PER_CORE = ROWS // N_CORES  # 8192
P = 128                          # SBUF partitions
T = ROWS_PER_CORE // P           # 64 subtiles per core
COLS = T * D                     # 16384 dram cols per partition
CHUNK_T = 16                     # subtiles per DMA chunk (2 MB / tensor)
N_CHUNKS = T // CHUNK_T

_CACHE: dict = {}


def _build_bass():
    f32 = mybir.dt.float32
    alu = mybir.AluOpType
    act = mybir.ActivationFunctionType

    nc = bass.Bass("TRN2", debug=False, num_devices=N_CORES)
    a_d = nc.dram_tensor("a", (P, COLS), f32, kind="ExternalInput").ap()
    b_d = nc.dram_tensor("b", (P, COLS), f32, kind="ExternalInput").ap()
    o_d = nc.dram_tensor("out", (P, T), f32, kind="ExternalOutput").ap()

    with tile.TileContext(nc) as tc:
        with (
            tc.tile_pool(name="stats", bufs=1) as stats_pool,
            tc.tile_pool(name="chunks", bufs=2) as chunk_pool,
            tc.tile_pool(name="scratch", bufs=4) as scratch_pool,
            tc.tile_pool(name="sqb", bufs=2) as sqb_pool,
            tc.tile_pool(name="fin", bufs=1) as fin_pool,
        ):
            p_t = stats_pool.tile([P, T], f32, tag="p")
            sa_t = stats_pool.tile([P, T], f32, tag="sa")
            sb_t = stats_pool.tile([P, T], f32, tag="sb")

            for ch in range(N_CHUNKS):
                c0 = ch * CHUNK_T * D
                c1 = (ch + 1) * CHUNK_T * D
                a_ch = chunk_pool.tile([P, CHUNK_T * D], f32, tag="a")
                b_ch = chunk_pool.tile([P, CHUNK_T * D], f32, tag="b")
                nc.sync.dma_start(a_ch[:], a_d[:, c0:c1])
                nc.sync.dma_start(b_ch[:], b_d[:, c0:c1])

                # Sb for the whole chunk: GPSIMD big elementwise square,
                # then one segmented DVE reduce over [P, CHUNK_T, D].
                sqb = sqb_pool.tile([P, CHUNK_T * D], f32, tag="sqb")
                nc.gpsimd.tensor_tensor(sqb[:], b_ch[:], b_ch[:], alu.mult)
                nc.vector.tensor_reduce(
                    sb_t[:, ch * CHUNK_T : (ch + 1) * CHUNK_T],
                    sqb[:].rearrange("p (t d) -> p t d", d=D),
                    axis=mybir.AxisListType.X,
                    op=alu.add,
                )

                for s in range(CHUNK_T):
                    t = ch * CHUNK_T + s
                    asub = a_ch[:, s * D : (s + 1) * D]
                    bsub = b_ch[:, s * D : (s + 1) * D]

                    # P: fused multiply + accum-reduce on DVE.
                    prod = scratch_pool.tile([P, D], f32, tag="prod")
                    nc.vector.scalar_tensor_tensor(
                        out=prod[:],
                        in0=asub,
                        scalar=0.0,
                        in1=bsub,
                        op0=alu.add,
                        op1=alu.mult,
                        accum_out=p_t[:, t : t + 1],
                    )

                    # Sa: fused square + accumulate on ACT.
                    scr = scratch_pool.tile([P, D], f32, tag="asq")
                    nc.scalar.activation(
                        scr[:], asub, act.Square, accum_out=sa_t[:, t : t + 1]
                    )

            # Finalize: out = P * sqrt(1 / (Sa * Sb)).
            denom = fin_pool.tile([P, T], f32, tag="denom")
            nc.vector.tensor_mul(denom[:], sa_t[:], sb_t[:])
            rec = fin_pool.tile([P, T], f32, tag="rec")
            nc.vector.reciprocal(rec[:], denom[:])
            rsq = fin_pool.tile([P, T], f32, tag="rsq")
            nc.scalar.activation(rsq[:], rec[:], act.Sqrt)
            out_t = fin_pool.tile([P, T], f32, tag="out")
            nc.vector.tensor_mul(out_t[:], p_t[:], rsq[:])
            nc.sync.dma_start(o_d[:], out_t[:])

    return nc


def _get_nc():
    if "nc" not in _CACHE:
        _CACHE["nc"] = _build_bass()
    return _CACHE["nc"]


def kernel(a: np.ndarray, b: np.ndarray) -> np.ndarray:
    a = np.ascontiguousarray(np.asarray(a, dtype=np.float32)).reshape(ROWS, D)
    b = np.ascontiguousarray(np.asarray(b, dtype=np.float32)).reshape(ROWS, D)

    in_maps = []
    for c in range(N_CORES):
        sl = slice(c * ROWS_PER_CORE, (c + 1) * ROWS_PER_CORE)
        in_maps.append(
            {"a": a[sl].reshape(P, COLS), "b": b[sl].reshape(P, COLS)}
        )

    nc = _get_nc()
    res = bass_utils.run_bass_kernel_spmd(nc, in_maps, core_ids=list(range(N_CORES)))
    out = np.concatenate(
        [res.results[c]["out"].reshape(ROWS_PER_CORE) for c in range(N_CORES)]
    )
    return out.reshape(B, N)


# revision 6
# speedup vs baseline: 1.2822x; 1.2822x over previous
"""Rowwise cosine-similarity kernel for Trainium2 (8 NeuronCores, SPMD).

Computes out[b, n] = sum_d(an * bn) where an, bn are L2-normalized rows of
a, b [16, 4096, 256] -> out [16, 4096].

Sharding: 65536 rows split across 8 cores (8192 rows/core). Per core the
row slice is viewed as [128 partitions, 64 subtiles, 256], i.e. row
p*64 + t lives at partition p, subtile t. Everything is contiguous DMA.

Per 256-wide subtile:
  P  = sum(a*b)  via DVE scalar_tensor_tensor (fused mul + accum-reduce)
  Sa = sum(a*a)  via ACT Square+accum_out or DVE stt (load-balanced)
  Sb = sum(b*b)  likewise
Finalize (batched [128, 64]): out = P * sqrt(1/(Sa*Sb)).
"""

import sys

if "/opt/trn_rl_repo" not in sys.path:
    sys.path.insert(0, "/opt/trn_rl_repo")

import numpy as np
import orjson

import concourse.bass as bass
import concourse.mybir as mybir
import concourse.tile as tile
from concourse import bass2jax, bass_utils

# ---------------------------------------------------------------------------
# Environment patches.
#
# 1. No cloud share in this sandbox: upload_artifacts would fail.
# 2. The walrus build here accepts at most ONE semaphore wait per
#    instruction; the Tile scheduler freely attaches several.  Post-process
#    the BIR before compiling: move surplus waits onto single-wait Drain
#    carrier instructions inserted just before the original instruction on
#    the same engine queue.
# ---------------------------------------------------------------------------

bass_utils.upload_artifacts = lambda tmpdir: ""

_MAX_WAITS = 1


def _split_bir_waits(bir_json: bytes) -> bytes:
    d = orjson.loads(bir_json)
    ctr = 0
    for fn in d.get("functions", []):
        for blk in fn.get("blocks", []):
            insts = blk.get("instructions")
            if not insts:
                continue
            out = []
            for inst in insts:
                si = inst.get("sync_info")
                waits = (si or {}).get("on_wait") or []
                if len(waits) > _MAX_WAITS:
                    surplus = waits[:-_MAX_WAITS]
                    si["on_wait"] = waits[-_MAX_WAITS:]
                    for i in range(0, len(surplus), _MAX_WAITS):
                        out.append(
                            {
                                "name": f"WSPL-{ctr}",
                                "opcode": "Drain",
                                "engine": inst["engine"],
                                "ins": [],
                                "outs": [],
                                "is_reset_sema": False,
                                "debug": inst.get("debug", 0),
                                "sync_info": {
                                    "on_wait": surplus[i : i + _MAX_WAITS],
                                    "on_update": [],
                                },
                            }
                        )
                        ctr += 1
                out.append(inst)
            blk["instructions"] = out
    return orjson.dumps(d)


_orig_compile_bir_kernel = bass_utils.compile_bir_kernel


def _patched_compile_bir_kernel(bir_json, tmpdir, neff_name="file.neff"):
    if isinstance(bir_json, str):
        bir_json = bir_json.encode()
    bir_json = _split_bir_waits(bir_json)
    return _orig_compile_bir_kernel(bir_json, tmpdir, neff_name=neff_name)


bass_utils.compile_bir_kernel = _patched_compile_bir_kernel
bass2jax.compile_bir_kernel = _patched_compile_bir_kernel

# ---------------------------------------------------------------------------
# Problem constants (hardcoded; kernel.py must be self-contained).
# ---------------------------------------------------------------------------

N_CORES = 8
B, N, D = 16, 4096, 256
ROWS = B * N                     # 65536
ROWS_PER_CORE = ROWS // N_CORES  # 8192
P = 128                          # SBUF partitions
T = ROWS_PER_CORE // P           # 64 subtiles per core
COLS = T * D                     # 16384 dram cols per partition
CHUNK_T = 16                     # subtiles per DMA chunk (2 MB / tensor)
N_CHUNKS = T // CHUNK_T

# Number of the 2*T square ops computed on DVE (rest on ACT); load balance.
# Measured per-op costs: DVE stt ~340 ns, ACT square+accum ~593 ns.
N_DVE_SQUARES = 58


def _square_on_dve(slot: int) -> bool:
    """Spread N_DVE_SQUARES of the 2*T square slots evenly onto the DVE."""
    return (slot * N_DVE_SQUARES) // (2 * T) != ((slot + 1) * N_DVE_SQUARES) // (
        2 * T
    )


_CACHE: dict = {}


def _build_bass():
    f32 = mybir.dt.float32
    alu = mybir.AluOpType
    act = mybir.ActivationFunctionType

    nc = bass.Bass("TRN2", debug=False, num_devices=N_CORES)
    a_d = nc.dram_tensor("a", (P, COLS), f32, kind="ExternalInput").ap()
    b_d = nc.dram_tensor("b", (P, COLS), f32, kind="ExternalInput").ap()
    o_d = nc.dram_tensor("out", (P, T), f32, kind="ExternalOutput").ap()

    with tile.TileContext(nc) as tc:
        with (
            tc.tile_pool(name="stats", bufs=1) as stats_pool,
            tc.tile_pool(name="chunks", bufs=2) as chunk_pool,
            tc.tile_pool(name="scratch", bufs=4) as scratch_pool,
            tc.tile_pool(name="sqb", bufs=2) as sqb_pool,
            tc.tile_pool(name="fin", bufs=1) as fin_pool,
        ):
            p_t = stats_pool.tile([P, T], f32, tag="p")
            sa_t = stats_pool.tile([P, T], f32, tag="sa")
            sb_t = stats_pool.tile([P, T], f32, tag="sb")

            for ch in range(N_CHUNKS):
                c0 = ch * CHUNK_T * D
                c1 = (ch + 1) * CHUNK_T * D
                a_ch = chunk_pool.tile([P, CHUNK_T * D], f32, tag="a")
                b_ch = chunk_pool.tile([P, CHUNK_T * D], f32, tag="b")
                nc.sync.dma_start(a_ch[:], a_d[:, c0:c1])
                nc.sync.dma_start(b_ch[:], b_d[:, c0:c1])

                for s in range(CHUNK_T):
                    t = ch * CHUNK_T + s
                    asub = a_ch[:, s * D : (s + 1) * D]
                    bsub = b_ch[:, s * D : (s + 1) * D]

                    # P: fused multiply + accum-reduce on DVE.
                    prod = scratch_pool.tile([P, D], f32, tag="prod")
                    nc.vector.scalar_tensor_tensor(
                        out=prod[:],
                        in0=asub,
                        scalar=0.0,
                        in1=bsub,
                        op0=alu.add,
                        op1=alu.mult,
                        accum_out=p_t[:, t : t + 1],
                    )

                    # Sa / Sb: fused square + accumulate, split DVE / ACT.
                    for which, sub, dst in ((0, asub, sa_t), (1, bsub, sb_t)):
                        if _square_on_dve(2 * t + which):
                            scr = scratch_pool.tile([P, D], f32, tag="dsq")
                            nc.vector.scalar_tensor_tensor(
                                out=scr[:],
                                in0=sub,
                                scalar=0.0,
                                in1=sub,
                                op0=alu.add,
                                op1=alu.mult,
                                accum_out=dst[:, t : t + 1],
                            )
                        else:
                            scr = scratch_pool.tile([P, D], f32, tag="asq")
                            nc.scalar.activation(
                                scr[:], sub, act.Square, accum_out=dst[:, t : t + 1]
                            )

            # Finalize: out = P * sqrt(1 / (Sa * Sb)).
            denom = fin_pool.tile([P, T], f32, tag="denom")
            nc.vector.tensor_mul(denom[:], sa_t[:], sb_t[:])
            rec = fin_pool.tile([P, T], f32, tag="rec")
            nc.vector.reciprocal(rec[:], denom[:])
            rsq = fin_pool.tile([P, T], f32, tag="rsq")
            nc.scalar.activation(rsq[:], rec[:], act.Sqrt)
            out_t = fin_pool.tile([P, T], f32, tag="out")
            nc.vector.tensor_mul(out_t[:], p_t[:], rsq[:])
            nc.sync.dma_start(o_d[:], out_t[:])

    return nc


def _get_nc():
    if "nc" not in _CACHE:
        _CACHE["nc"] = _build_bass()
    return _CACHE["nc"]


def kernel(a: np.ndarray, b: np.ndarray) -> np.ndarray:
    a = np.ascontiguousarray(np.asarray(a, dtype=np.float32)).reshape(ROWS, D)
    b = np.ascontiguousarray(np.asarray(b, dtype=np.float32)).reshape(ROWS, D)

    in_maps = []
    for c in range(N_CORES):
        sl = slice(c * ROWS_PER_CORE, (c + 1) * ROWS_PER_CORE)
        in_maps.append(
            {"a": a[sl].reshape(P, COLS), "b": b[sl].reshape(P, COLS)}
        )

    nc = _get_nc()
    res = bass_utils.run_bass_kernel_spmd(nc, in_maps, core_ids=list(range(N_CORES)))
    out = np.concatenate(
        [res.results[c]["out"].reshape(ROWS_PER_CORE) for c in range(N_CORES)]
    )
    return out.reshape(B, N)


# revision 7
# speedup vs baseline: 1.4228x; 1.1096x over previous
"""Rowwise cosine-similarity kernel for Trainium2 (8 NeuronCores, SPMD).

Computes out[b, n] = sum_d(an * bn) where an, bn are L2-normalized rows of
a, b [16, 4096, 256] -> out [16, 4096].

Sharding: 65536 rows split across 8 cores (8192 rows/core). Per core the
row slice is viewed as [128 partitions, 64 subtiles, 256], i.e. row
p*64 + t lives at partition p, subtile t. Everything is contiguous DMA.

Per 256-wide subtile:
  P  = sum(a*b)  via DVE scalar_tensor_tensor (fused mul + accum-reduce)
  Sa = sum(a*a)  via ACT Square+accum_out or DVE stt (load-balanced)
  Sb = sum(b*b)  likewise
Finalize (batched [128, 64]): out = P * sqrt(1/(Sa*Sb)).
"""

import sys

if "/opt/trn_rl_repo" not in sys.path:
    sys.path.insert(0, "/opt/trn_rl_repo")

import numpy as np
import orjson

import concourse.bass as bass
import concourse.mybir as mybir
import concourse.tile as tile
from concourse import bass2jax, bass_utils

# ---------------------------------------------------------------------------
# Environment patches.
#
# 1. No cloud share in this sandbox: upload_artifacts would fail.
# 2. The walrus build here accepts at most ONE semaphore wait per
#    instruction; the Tile scheduler freely attaches several.  Post-process
#    the BIR before compiling: move surplus waits onto single-wait Drain
#    carrier instructions inserted just before the original instruction on
#    the same engine queue.
# ---------------------------------------------------------------------------

bass_utils.upload_artifacts = lambda tmpdir: ""

_MAX_WAITS = 1


def _split_bir_waits(bir_json: bytes) -> bytes:
    d = orjson.loads(bir_json)
    ctr = 0
    for fn in d.get("functions", []):
        for blk in fn.get("blocks", []):
            insts = blk.get("instructions")
            if not insts:
                continue
            out = []
            for inst in insts:
                si = inst.get("sync_info")
                waits = (si or {}).get("on_wait") or []
                if len(waits) > _MAX_WAITS:
                    surplus = waits[:-_MAX_WAITS]
                    si["on_wait"] = waits[-_MAX_WAITS:]
                    for i in range(0, len(surplus), _MAX_WAITS):
                        out.append(
                            {
                                "name": f"WSPL-{ctr}",
                                "opcode": "Drain",
                                "engine": inst["engine"],
                                "ins": [],
                                "outs": [],
                                "is_reset_sema": False,
                                "debug": inst.get("debug", 0),
                                "sync_info": {
                                    "on_wait": surplus[i : i + _MAX_WAITS],
                                    "on_update": [],
                                },
                            }
                        )
                        ctr += 1
                out.append(inst)
            blk["instructions"] = out
    return orjson.dumps(d)


_orig_compile_bir_kernel = bass_utils.compile_bir_kernel


def _patched_compile_bir_kernel(bir_json, tmpdir, neff_name="file.neff"):
    if isinstance(bir_json, str):
        bir_json = bir_json.encode()
    bir_json = _split_bir_waits(bir_json)
    return _orig_compile_bir_kernel(bir_json, tmpdir, neff_name=neff_name)


bass_utils.compile_bir_kernel = _patched_compile_bir_kernel
bass2jax.compile_bir_kernel = _patched_compile_bir_kernel

# ---------------------------------------------------------------------------
# Problem constants (hardcoded; kernel.py must be self-contained).
# ---------------------------------------------------------------------------

N_CORES = 8
B, N, D = 16, 4096, 256
ROWS = B * N                     # 65536
ROWS_PER_CORE = ROWS // N_CORES  # 8192
P = 128                          # SBUF partitions
T = ROWS_PER_CORE // P           # 64 subtiles per core
COLS = T * D                     # 16384 dram cols per partition
CHUNK_T = 8                      # subtiles per DMA chunk (1 MB / tensor)
N_CHUNKS = T // CHUNK_T

# Number of the 2*T square ops computed on DVE (rest on ACT); load balance.
# Measured per-op costs: DVE stt ~340 ns, ACT square+accum ~593 ns.
N_DVE_SQUARES = 58


def _square_on_dve(slot: int) -> bool:
    """Spread N_DVE_SQUARES of the 2*T square slots evenly onto the DVE."""
    return (slot * N_DVE_SQUARES) // (2 * T) != ((slot + 1) * N_DVE_SQUARES) // (
        2 * T
    )


_CACHE: dict = {}


def _build_bass():
    f32 = mybir.dt.float32
    alu = mybir.AluOpType
    act = mybir.ActivationFunctionType

    nc = bass.Bass("TRN2", debug=False, num_devices=N_CORES)
    a_d = nc.dram_tensor("a", (P, COLS), f32, kind="ExternalInput").ap()
    b_d = nc.dram_tensor("b", (P, COLS), f32, kind="ExternalInput").ap()
    o_d = nc.dram_tensor("out", (P, T), f32, kind="ExternalOutput").ap()

    with tile.TileContext(nc) as tc:
        with (
            tc.tile_pool(name="stats", bufs=1) as stats_pool,
            tc.tile_pool(name="chunks", bufs=3) as chunk_pool,
            tc.tile_pool(name="scratch", bufs=4) as scratch_pool,
            tc.tile_pool(name="sqb", bufs=2) as sqb_pool,
            tc.tile_pool(name="fin", bufs=1) as fin_pool,
        ):
            p_t = stats_pool.tile([P, T], f32, tag="p")
            sa_t = stats_pool.tile([P, T], f32, tag="sa")
            sb_t = stats_pool.tile([P, T], f32, tag="sb")

            for ch in range(N_CHUNKS):
                c0 = ch * CHUNK_T * D
                c1 = (ch + 1) * CHUNK_T * D
                a_ch = chunk_pool.tile([P, CHUNK_T * D], f32, tag="a")
                b_ch = chunk_pool.tile([P, CHUNK_T * D], f32, tag="b")
                nc.sync.dma_start(a_ch[:], a_d[:, c0:c1])
                nc.sync.dma_start(b_ch[:], b_d[:, c0:c1])

                for s in range(CHUNK_T):
                    t = ch * CHUNK_T + s
                    asub = a_ch[:, s * D : (s + 1) * D]
                    bsub = b_ch[:, s * D : (s + 1) * D]

                    # P: fused multiply + accum-reduce on DVE.
                    prod = scratch_pool.tile([P, D], f32, tag="prod")
                    nc.vector.scalar_tensor_tensor(
                        out=prod[:],
                        in0=asub,
                        scalar=0.0,
                        in1=bsub,
                        op0=alu.add,
                        op1=alu.mult,
                        accum_out=p_t[:, t : t + 1],
                    )

                    # Sa / Sb: fused square + accumulate, split DVE / ACT.
                    for which, sub, dst in ((0, asub, sa_t), (1, bsub, sb_t)):
                        if _square_on_dve(2 * t + which):
                            scr = scratch_pool.tile([P, D], f32, tag="dsq")
                            nc.vector.scalar_tensor_tensor(
                                out=scr[:],
                                in0=sub,
                                scalar=0.0,
                                in1=sub,
                                op0=alu.add,
                                op1=alu.mult,
                                accum_out=dst[:, t : t + 1],
                            )
                        else:
                            scr = scratch_pool.tile([P, D], f32, tag="asq")
                            nc.scalar.activation(
                                scr[:], sub, act.Square, accum_out=dst[:, t : t + 1]
                            )

            # Finalize: out = P * sqrt(1 / (Sa * Sb)).
            denom = fin_pool.tile([P, T], f32, tag="denom")
            nc.vector.tensor_mul(denom[:], sa_t[:], sb_t[:])
            rec = fin_pool.tile([P, T], f32, tag="rec")
            nc.vector.reciprocal(rec[:], denom[:])
            rsq = fin_pool.tile([P, T], f32, tag="rsq")
            nc.scalar.activation(rsq[:], rec[:], act.Sqrt)
            out_t = fin_pool.tile([P, T], f32, tag="out")
            nc.vector.tensor_mul(out_t[:], p_t[:], rsq[:])
            nc.sync.dma_start(o_d[:], out_t[:])

    return nc


def _get_nc():
    if "nc" not in _CACHE:
        _CACHE["nc"] = _build_bass()
    return _CACHE["nc"]


def kernel(a: np.ndarray, b: np.ndarray) -> np.ndarray:
    a = np.ascontiguousarray(np.asarray(a, dtype=np.float32)).reshape(ROWS, D)
    b = np.ascontiguousarray(np.asarray(b, dtype=np.float32)).reshape(ROWS, D)

    in_maps = []
    for c in range(N_CORES):
        sl = slice(c * ROWS_PER_CORE, (c + 1) * ROWS_PER_CORE)
        in_maps.append(
            {"a": a[sl].reshape(P, COLS), "b": b[sl].reshape(P, COLS)}
        )

    nc = _get_nc()
    res = bass_utils.run_bass_kernel_spmd(nc, in_maps, core_ids=list(range(N_CORES)))
    out = np.concatenate(
        [res.results[c]["out"].reshape(ROWS_PER_CORE) for c in range(N_CORES)]
    )
    return out.reshape(B, N)
